# revision 3
# baseline (speedup 1.0000x reference)
"""BiGCN (bidirectional 2-layer GCN over many small graphs) on 8 Trainium2 cores.

v4: graph-pair batching everywhere. Two lessons from v3's trace drove
this: (1) the PE has a ~110ns per-matmul floor, so N=128 matmuls are
instruction-bound; (2) DVE/ACT ops cost ~250ns fixed + ~0.7ns/elem, so
per-graph 512-elem ops leave both engines ~45% busy on overhead.

Graphs are processed in PAIRS (2i, 2i+1). The aggregation matmuls use a
zero-padded DoubleRow trick: with fp8 operands, k-tile0 = Y_e vs
rhs [Bt_e | 0] and k-tile1 = Y_o vs [0 | Bt_o] makes ONE matmul compute
both graphs' aggregations into disjoint output columns (N=256 streams
instead of two N=128 — same cycles, half the instructions). All PSUM
evacuations/reductions run at [128, 4, 256] pair granularity.

Accuracy (sim ~6-7e-3 vs the 2e-2 gate): X/W1/W2/Y/Z/B/h all single
fp8e4m3. X and Y quantization errors are independent across nodes and
average out of the 128-node mean readout; W1's systematic error washes
out because X is zero-mean. The un-averaged root columns are computed
EXACTLY on the host (~300 MFLOP numpy) and spliced into the output.
W2's would-be-systematic error stays small because Z also feeds back
through zero-mean-ish aggregation; sim confirms.

Per pair P (4-stage software pipeline):
  A : Y_g = X_g W1        6 DoubleRow fp8 mm (3 per graph), N=512 -> psY2
      yn2 = fp8(psY2)     2 scalar copies (one per graph)
  B1: hT = relu(B Y)^T    4 pair-DR mm, N=256 -> psH2; 1 scalar relu->fp8
  B2: Z' = h W2 + 1(x)rv  4 graph-DR mm N=256 + 2 rank-1 (K=1) N=512
      zn2 = fp8(psZ2)     1 vector cast
  B3: pre2^T = (B Z')^T   4 pair-DR mm, N=256 -> ps22
      h2 = relu(pre2)     split scalar/vector; 1 vector pair-reduce
Readout: rc [128,4,2,16] f32 -> one DMA; host does transpose + /128.
"""

import numpy as np
import ml_dtypes

import concourse.bass as bass
import concourse.tile as tile
from concourse import bacc, mybir
from concourse.bass_utils import run_bass_kernel_spmd

# Problem shape (fixed by the task)
N_GRAPHS = 256
N_PER_G = 128
IN_FEATS = 768
H_FEATS = 256
N_CORES = 8
G_PER_CORE = N_GRAPHS // N_CORES            # 32
P_PER_CORE = G_PER_CORE // 2                # 16 graph pairs
NODES_PER_CORE = G_PER_CORE * N_PER_G       # 4096
KCH = IN_FEATS // 128                       # 6 feature chunks

F8 = mybir.dt.float8e4
F16 = mybir.dt.float16
F32 = mybir.dt.float32
AF = mybir.ActivationFunctionType
OP = mybir.AluOpType
DR = mybir.MatmulPerfMode.DoubleRow
NPF8 = ml_dtypes.float8_e4m3


# ----------------------------------------------------------------------------
# Device program (SPMD; one core's shard)
# ----------------------------------------------------------------------------

def build_program(has_b1=False):
    nc = bacc.Bacc("TRN2", target_bir_lowering=False, debug=False,
                   num_devices=N_CORES)

    def din(name, shape, dt):
        return nc.dram_tensor(name, shape, dt, kind="ExternalInput").ap()

    # x8[p, P, go, k, d] = X^T chunk; aux = zero-padded B^T pair tiles
    x8 = din("x8", [128, P_PER_CORE, 2, KCH, 128], F8)
    aux = din("aux", [128, P_PER_CORE, 2, 2, 256], F8)
    rvb = din("rvb", [128, P_PER_CORE, 2, 2 * H_FEATS], F16)
    w1a = din("w1a", [128, KCH, 2 * H_FEATS], F8)
    w2a = din("w2a", [128, 4, H_FEATS], F8)
    b1c = din("b1c", [128, 4], F32) if has_b1 else None
    out = nc.dram_tensor("out", [128, 4, 2, P_PER_CORE], F32,
                         kind="ExternalOutput").ap()

    with tile.TileContext(nc) as tc:
        with (
            tc.tile_pool(name="const", bufs=1) as const,
            tc.tile_pool(name="x8p", bufs=4) as x8p,
            tc.tile_pool(name="auxp", bufs=7) as auxp,
            tc.tile_pool(name="ya", bufs=2) as yap,
            tc.tile_pool(name="ha", bufs=2) as hap,
            tc.tile_pool(name="za", bufs=2) as zap,
            tc.tile_pool(name="rvp", bufs=7) as rvp,
            tc.tile_pool(name="h2a", bufs=2) as h2ap,
            # half-size tiles with bufs=2: same bank count as full-size
            # bufs=1, but the rotation double-buffers each half so the
            # PE->evac->PE round-trip gets a full iteration of slack
            tc.tile_pool(name="psY", bufs=2, space="PSUM") as psY,
            tc.tile_pool(name="psH", bufs=2, space="PSUM") as psH,
            tc.tile_pool(name="psZ", bufs=2, space="PSUM") as psZ,
            tc.tile_pool(name="ps2", bufs=2, space="PSUM") as ps2p,
        ):
            w1a_sb = const.tile([128, KCH, 2 * H_FEATS], F8)
            w2a_sb = const.tile([128, 4, H_FEATS], F8)
            if has_b1:
                b1c_sb = const.tile([128, 4], F32)

            rc_m = const.tile([128, 4, 2, P_PER_CORE], F32)

            dmat = {}

            def stage_dma(p):
                x8_t = x8p.tile([128, 2, KCH, 128], F8, tag="x8")
                aux_t = auxp.tile([128, 2, 2, 256], F8, tag="aux")
                rvb_t = rvp.tile([128, 2, 2 * H_FEATS], F16, tag="rvb")
                nc.sync.dma_start(x8_t[:], x8[:, p])
                nc.gpsimd.dma_start(aux_t[:], aux[:, p])
                # sync is a HWDGE ring with queue headroom; keep the 4MB rvb
                # stream off the slow software-dynamic (gpsimd) path and its
                # ~0.6us trigger cost off the busy scalar queue
                nc.sync.dma_start(rvb_t[:], rvb[:, p])
                dmat[p] = (x8_t, aux_t, rvb_t)

            def mms_A(p):
                x8_t, aux_t, rvb_t = dmat[p]
                ps_y = [psY.tile([128, 2 * H_FEATS], F32, tag="y", name="ps_y")
                        for _ in (0, 1)]
                dmat[p] = (x8_t, aux_t, rvb_t, ps_y)
                mm = []
                for go in (0, 1):
                    for kp in range(KCH // 2):
                        mm.append(lambda go=go, kp=kp: nc.tensor.matmul(
                            ps_y[go][:],
                            x8_t[:, go, 2 * kp:2 * kp + 2, :],
                            w1a_sb[:, 2 * kp:2 * kp + 2, :],
                            start=(kp == 0), stop=(kp == KCH // 2 - 1),
                            perf_mode=DR))
                return mm

            def post_A(p):
                x8_t, aux_t, rvb_t, ps_y = dmat[p]
                yn = yap.tile([128, 2, 2 * H_FEATS], F8, tag="yn")
                nc.scalar.copy(yn[:, 0, :], ps_y[0][:])
                nc.scalar.copy(yn[:, 1, :], ps_y[1][:])
                dmat[p] = (aux_t, rvb_t, yn)

            def mms_B1(p):
                aux_t, rvb_t, yn = dmat[p]
                ps_h = [psH.tile([128, 2, 2 * N_PER_G], F32, tag="h", name="ps_h")
                        for _ in (0, 1)]
                dmat[p] = (aux_t, rvb_t, ps_h)
                mm = []
                for br in (0, 1):
                    for hc in (0, 1):
                        c0 = br * 256 + hc * 128
                        mm.append(lambda hc=hc, c0=c0, br=br: nc.tensor.matmul(
                            ps_h[br][:, hc, :], yn[:, :, c0:c0 + 128],
                            aux_t[:, br, :, :], perf_mode=DR))
                return mm

            def post_B1(p):
                aux_t, rvb_t, ps_h = dmat[p]
                hT = hap.tile([128, 4, 2 * N_PER_G], F8, tag="hT")
                if has_b1:
                    for br in (0, 1):
                        for hc in (0, 1):
                            j = br * 2 + hc
                            nc.scalar.activation(hT[:, j, :],
                                                 ps_h[br][:, hc, :],
                                                 AF.Relu,
                                                 bias=b1c_sb[:, j:j + 1])
                else:
                    nc.scalar.activation(hT[:, 0:2, :], ps_h[0][:], AF.Relu)
                    nc.scalar.activation(hT[:, 2:4, :], ps_h[1][:], AF.Relu)
                dmat[p] = (aux_t, rvb_t, hT)

            def mms_B2(p):
                aux_t, rvb_t, hT = dmat[p]
                ps_z = [psZ.tile([128, 2 * H_FEATS], F32, tag="z", name="ps_z")
                        for _ in (0, 1)]
                dmat[p] = (aux_t, rvb_t, ps_z)
                mm = []
                for go in (0, 1):
                    for br in (0, 1):
                        mm.append(lambda go=go, br=br: nc.tensor.matmul(
                            ps_z[go][:, br * 256:(br + 1) * 256],
                            hT[:, 2 * br:2 * br + 2, go * 128:(go + 1) * 128],
                            w2a_sb[:, 2 * br:2 * br + 2, :],
                            perf_mode=DR))
                return mm

            def post_B2(p):
                aux_t, rvb_t, ps_z = dmat[p]
                zn = zap.tile([128, 2, 2 * H_FEATS], F8, tag="zn")
                # Z' = Z + 1 (x) rv folded into the PSUM evacuation
                nc.vector.tensor_tensor(zn[:, 0, :], ps_z[0][:],
                                        rvb_t[:, 0, :], OP.add)
                nc.vector.tensor_tensor(zn[:, 1, :], ps_z[1][:],
                                        rvb_t[:, 1, :], OP.add)
                dmat[p] = (aux_t, zn)

            def mms_B3(p):
                aux_t, zn = dmat[p]
                ps_2 = [ps2p.tile([128, 2, 2 * N_PER_G], F32, tag="p2", name="ps_2")
                        for _ in (0, 1)]
                dmat[p] = (aux_t, zn, ps_2)
                mm = []
                for br in (0, 1):
                    for zc in (0, 1):
                        c0 = br * 256 + zc * 128
                        mm.append(lambda zc=zc, c0=c0, br=br: nc.tensor.matmul(
                            ps_2[br][:, zc, :], zn[:, :, c0:c0 + 128],
                            aux_t[:, br, :, :], perf_mode=DR))
                return mm

            def post_B3_relu(p):
                aux_t, zn, ps_2 = dmat.pop(p)
                h2a = h2ap.tile([128, 2, 2 * N_PER_G], F16, tag="h2a")
                h2b = h2ap.tile([128, 2, 2 * N_PER_G], F16, tag="h2b")
                nc.vector.tensor_scalar(h2a[:], ps_2[0][:],
                                        0.0, None, OP.max)
                nc.scalar.activation(h2b[:], ps_2[1][:], AF.Relu)
                dmat[(p, "h2")] = (h2a, h2b)

            def post_B3_reduce(p):
                h2a, h2b = dmat.pop((p, "h2"))
                nc.vector.tensor_reduce(
                    rc_m[:, 0:2, :, p:p + 1],
                    h2a[:].rearrange("p j (go d) -> p j go d", go=2),
                    mybir.AxisListType.X, OP.add)
                nc.vector.tensor_reduce(
                    rc_m[:, 2:4, :, p:p + 1],
                    h2b[:].rearrange("p j (go d) -> p j go d", go=2),
                    mybir.AxisListType.X, OP.add)

            # ---- software-pipelined main loop ------------------------------
            nc.scalar.dma_start(w1a_sb[:, 0:2, :], w1a[:, 0:2, :])
            nc.gpsimd.dma_start(w1a_sb[:, 2:KCH, :], w1a[:, 2:KCH, :])
            nc.gpsimd.dma_start(w2a_sb[:], w2a)
            if has_b1:
                nc.gpsimd.dma_start(b1c_sb[:], b1c)
            for p in range(2):
                stage_dma(p)
            P = P_PER_CORE
            for it in range(P + 3):
                if it + 2 < P:
                    stage_dma(it + 2)
                tA = mms_A(it) if it < P else []
                tB1 = mms_B1(it - 1) if 0 <= it - 1 < P else []
                tB2 = mms_B2(it - 2) if 0 <= it - 2 < P else []
                tB3 = mms_B3(it - 3) if 0 <= it - 3 < P else []
                # B3/B2 first (their inputs settled a full iteration ago),
                # B1 next (yn lands during B3/B2), A last (psY WAR on the
                # yn copy issued at the end of the previous iteration)
                for f in tB3 + tB2 + tB1 + tA:
                    f()
                # posts in data-readiness order per engine queue; the zn add
                # gates next iteration's first PE block (B3), so it must not
                # queue behind the non-gating reduces on the vector engine
                if 0 <= it - 3 < P:
                    post_B3_relu(it - 3)
                if 0 <= it - 2 < P:
                    post_B2(it - 2)
                if 0 <= it - 3 < P:
                    post_B3_reduce(it - 3)
                if 0 <= it - 1 < P:
                    post_B1(it - 1)
                if it < P:
                    post_A(it)
            nc.sync.dma_start(out[:], rc_m[:])

    nc.compile()
    return nc


# ----------------------------------------------------------------------------
# Host-side prep: normalized adjacency, fp8 quant, root readout, sharding
# ----------------------------------------------------------------------------

def _prep(inputs, w1_td, b1_td, w2_td, b2_td, w1_bu, b1_bu, w2_bu, b2_bu,
          td_src, td_dst, bu_src, bu_dst, nodes_per_graph):
    n = int(nodes_per_graph)
    X = np.asarray(inputs, np.float32)
    N, F = X.shape
    G = N // n
    assert (n, G, F) == (N_PER_G, N_GRAPHS, IN_FEATS), \
        f"unexpected shapes {X.shape} n={n}"

    def build(src, dst):
        src = np.asarray(src, np.int64)
        dst = np.asarray(dst, np.int64)
        g = src // n
        if not np.array_equal(dst // n, g):
            raise ValueError("cross-graph edge; contiguous sharding invalid")
        At = np.zeros((G, n, n), np.float32)   # At[g, src, dst] = A[dst, src]
        np.add.at(At, (g, src - g * n, dst - g * n), 1.0)
        deg = At.sum(axis=1)                   # in-degree per dst
        with np.errstate(divide="ignore"):
            norm = 1.0 / np.sqrt(deg)
        norm[~np.isfinite(norm)] = 0.0
        Bt = norm[:, :, None] * At * norm[:, None, :]  # B^T[g, src, dst]
        return Bt

    Bt_td = build(td_src, td_dst)
    Bt_bu = build(bu_src, bu_dst)

    w1_td = np.asarray(w1_td, np.float32)
    w1_bu = np.asarray(w1_bu, np.float32)
    w1p = np.concatenate([w1_td, w1_bu], axis=1)            # [768, 512]
    w2_td = np.asarray(w2_td, np.float32)
    w2_bu = np.asarray(w2_bu, np.float32)
    w2hs = np.concatenate([w2_td[:H_FEATS], w2_bu[:H_FEATS]], axis=0)
    roots = X[::n]                              # [G, 768]
    rva = np.concatenate([roots @ w2_td[H_FEATS:],
                          roots @ w2_bu[H_FEATS:]], axis=1)  # [G, 512]
    b1 = np.stack([np.asarray(b1_td, np.float32), np.asarray(b1_bu, np.float32)])
    b2 = np.stack([np.asarray(b2_td, np.float32), np.asarray(b2_bu, np.float32)])
    has_b1 = bool(np.any(b1 != 0))
    assert not np.any(b2 != 0), "device fast path assumes zero layer-2 bias"
    b1cols = np.ascontiguousarray(b1.reshape(4, 128).T)

    # ---- exact host root readout (error never averages there) -------------
    Xg = X.reshape(G, n, F)
    root_td = np.einsum('gs,gsf->gf', Bt_td[:, :, 0], Xg) @ w1_td + b1[0]
    root_bu = np.einsum('gs,gsf->gf', Bt_bu[:, :, 0], Xg) @ w1_bu + b1[1]
    root_td = np.maximum(root_td, 0.0)
    root_bu = np.maximum(root_bu, 0.0)

    # ---- fp8 weights, device layouts --------------------------------------
    w1a = np.ascontiguousarray(
        w1p.reshape(KCH, 128, 2 * H_FEATS).transpose(1, 0, 2)).astype(NPF8)
    w2a = np.ascontiguousarray(
        w2hs.reshape(4, 128, H_FEATS).transpose(1, 0, 2)).astype(NPF8)

    in_maps = []
    for c in range(N_CORES):
        gsl = slice(c * G_PER_CORE, (c + 1) * G_PER_CORE)
        nsl = slice(c * NODES_PER_CORE, (c + 1) * NODES_PER_CORE)
        # x8[p, P, go, k, d] = X[off + (2P+go)*128 + d, k*128 + p]
        Xc = X[nsl].reshape(P_PER_CORE, 2, 128, KCH, 128)
        x8c = np.ascontiguousarray(Xc.transpose(4, 0, 1, 3, 2)).astype(NPF8)
        # zero-padded B^T pair tiles: [p(src), P, br, ktile, dst-pair]
        auxc = np.zeros((128, P_PER_CORE, 2, 2, 256), np.float32)
        for br, Bt in ((0, Bt_td), (1, Bt_bu)):
            Bc = Bt[gsl].reshape(P_PER_CORE, 2, n, n)   # [P, go, src, dst]
            auxc[:, :, br, 0, 0:128] = Bc[:, 0].transpose(1, 0, 2)
            auxc[:, :, br, 1, 128:256] = Bc[:, 1].transpose(1, 0, 2)
        rvbc = np.broadcast_to(
            rva[gsl].reshape(P_PER_CORE, 2, 2 * H_FEATS)[None],
            (128, P_PER_CORE, 2, 2 * H_FEATS))
        m = {
            "x8": x8c,
            "aux": auxc.astype(NPF8),
            "rvb": np.ascontiguousarray(rvbc).astype(np.float16),
            "w1a": w1a,
            "w2a": w2a,
        }
        if has_b1:
            m["b1c"] = b1cols
        in_maps.append(m)
    return in_maps, has_b1, root_td, root_bu


_PROG = {}


def _get_program(has_b1):
    if has_b1 not in _PROG:
        _PROG[has_b1] = build_program(has_b1)
    return _PROG[has_b1]


def kernel(trace=False, tmpdir=None, _return_raw=False, **inputs):
    in_maps, has_b1, root_td, root_bu = _prep(**inputs)
    nc = _get_program(has_b1)
    res = run_bass_kernel_spmd(nc, in_maps, list(range(N_CORES)),
                               trace=trace, tmpdir=tmpdir)
    out = np.empty((N_GRAPHS, 4 * H_FEATS), np.float32)
    for c in range(N_CORES):
        rc = res.results[c]["out"]              # [128p, 4j, 2go, 16P]
        # mean[g=2P+go, br*256 + zc*128 + p] = rc[p, br*2+zc, go, P] / 128
        mean = rc.reshape(128, 2, 2, 2, P_PER_CORE).transpose(4, 3, 1, 2, 0)
        mean = mean.reshape(G_PER_CORE, 2 * H_FEATS) * (1.0 / N_PER_G)
        gs = slice(c * G_PER_CORE, (c + 1) * G_PER_CORE)
        out[gs, 0:256] = mean[:, 0:256]
        out[gs, 512:768] = mean[:, 256:512]
    out[:, 256:512] = root_td
    out[:, 768:1024] = root_bu
    if _return_raw:
        return out, res
    return out
